# revision 14
# baseline (speedup 1.0000x reference)
"""Trainium2 Bass kernel: 2D positional encoding via embedding lookup.

Computes out[b, s, :] = x[b, s, :] + concat(pe_h[cy[b,s]], pe_w[cx[b,s]])
where cy/cx are integer grid indices derived from box centers (the TRN jax
backend's float->int cast rounds to nearest even; we reproduce that exactly
with precomputed per-row comparison thresholds).

Sharding: data-parallel over batch (8 batches -> 8 NeuronCores), pe tables
replicated. Per core the gather is done ON-CHIP: the [100, 512] tables live
in SBUF (split hi/mid/lo into three bf16 planes whose fp32 sum reconstructs
the table bit-exactly) and rows are selected with a one-hot bf16 matmul on
the tensor engine, so HBM traffic is just x-in + boxes + out.
"""

import os
import sys

for _p in ("/opt/trn_rl_repo", "/root/.axon_site/_ro/trn_rl_repo"):
    if os.path.isdir(_p) and _p not in sys.path:
        sys.path.insert(0, _p)

import ml_dtypes
import numpy as np

import concourse.bacc as bacc
import concourse.mybir as mybir
from concourse.bass_utils import run_bass_kernel_spmd
from concourse.tile import TileContext

B = 8
S = 4096
D = 1024
HALF = D // 2  # 512
NCLASS = 100
P = 128
NTILES = S // P  # 32
NPAIRS = NTILES // 2  # 16
F32 = mybir.dt.float32
BF16 = mybir.dt.bfloat16

_NC = None


def _emit(tc, x, boxes, ident_d, tlo_d, thi_d, tabs_d, out):
    nc = tc.nc
    with (
        tc.tile_pool(name="const", bufs=1) as const,
        tc.tile_pool(name="xp", bufs=12) as xp,
        tc.tile_pool(name="cmp", bufs=2) as cmp_pool,
        tc.tile_pool(name="bcp", bufs=2, space="PSUM") as bcp,
        tc.tile_pool(name="gp", bufs=3, space="PSUM") as gp,
    ):
        # identity + boxes first on the fast HWDGE queue (contiguous loads:
        # boxes lands as [32 tiles, 128 tokens, 4] with 2KB descriptors), so
        # the compute pipeline starts within ~2us.
        identity = const.tile([P, P], F32)
        nc.sync.dma_start(out=identity[:], in_=ident_d)
        bx = const.tile([NTILES, P, 4], F32)
        nc.sync.dma_start(out=bx[:], in_=boxes.rearrange("(c p) e -> c p e", c=NTILES))

        tlo = const.tile([NCLASS, 1], F32)
        nc.sync.dma_start(out=tlo[:], in_=tlo_d)
        thi = const.tile([NCLASS, 1], F32)
        nc.sync.dma_start(out=thi[:], in_=thi_d)

        # six bf16 table planes: [h_hi, h_mid, h_lo, w_hi, w_mid, w_lo]
        tabs = const.tile([NCLASS, 6, HALF], BF16)
        nc.sync.dma_start(out=tabs[:], in_=tabs_d)

        # center values in row form: vrow[c, t] = v(token c*128+t), computed
        # as (lo + hi) * 49.5 -- same fp32 rounding as the reference's
        # ((lo + hi) * 0.5) * 99.0 (the *0.5 step is exact).
        vyr = const.tile([NTILES, P], F32)
        vxr = const.tile([NTILES, P], F32)
        nc.vector.tensor_tensor(
            out=vyr[:], in0=bx[:, :, 1], in1=bx[:, :, 3], op=mybir.AluOpType.add
        )
        nc.vector.tensor_scalar_mul(vyr[:], vyr[:], 49.5)
        nc.vector.tensor_tensor(
            out=vxr[:], in0=bx[:, :, 0], in1=bx[:, :, 2], op=mybir.AluOpType.add
        )
        nc.vector.tensor_scalar_mul(vxr[:], vxr[:], 49.5)

        # one-time transpose to column form: vcols[:, c] = vy col of tile c,
        # vcols[:, NTILES + c] = vx col of tile c.
        vt_ps = bcp.tile([P, 2 * NTILES], F32, tag="bc")
        nc.tensor.transpose(
            out=vt_ps[:, 0:NTILES], in_=vyr[:], identity=identity[0:NTILES, 0:NTILES]
        )
        nc.tensor.transpose(
            out=vt_ps[:, NTILES : 2 * NTILES],
            in_=vxr[:],
            identity=identity[0:NTILES, 0:NTILES],
        )
        vcols = const.tile([P, 2 * NTILES], F32)
        nc.vector.tensor_copy(vcols[:], vt_ps[:])

        for g in range(NPAIRS):
            c0 = 2 * g
            # Broadcast the pair's y/x values across NCLASS partitions via PE
            # transposes of free-dim-broadcast columns: bc[:, k*128:(k+1)*128]
            # = [vy(c0) | vx(c0) | vy(c0+1) | vx(c0+1)] broadcast rows.
            bc = bcp.tile([NCLASS, 4 * P], F32, tag="bc")
            for i in range(4):
                col = (c0 + i // 2) + (0 if i % 2 == 0 else NTILES)
                nc.tensor.transpose(
                    out=bc[:, i * P : (i + 1) * P],
                    in_=vcols[:, col : col + 1].to_broadcast([P, NCLASS]),
                    identity=identity[:],
                )
            # one-hot row k = (v >= tlo[k]) - (v >= thi[k]); exact {0,1} in bf16
            ge1 = cmp_pool.tile([NCLASS, 4 * P], F32, tag="ge1")
            nc.vector.tensor_scalar(
                out=ge1[:], in0=bc[:], scalar1=tlo[:], scalar2=None,
                op0=mybir.AluOpType.is_ge,
            )
            ge2 = cmp_pool.tile([NCLASS, 4 * P], F32, tag="ge2")
            nc.vector.tensor_scalar(
                out=ge2[:], in0=bc[:], scalar1=thi[:], scalar2=None,
                op0=mybir.AluOpType.is_ge,
            )
            oh = cmp_pool.tile([NCLASS, 4 * P], BF16, tag="oh")
            nc.vector.tensor_tensor(
                out=oh[:], in0=ge1[:], in1=ge2[:], op=mybir.AluOpType.subtract
            )

            for j in range(2):  # the two tiles of this pair
                c = c0 + j
                ohy = oh[:, (2 * j) * P : (2 * j + 1) * P]
                ohx = oh[:, (2 * j + 1) * P : (2 * j + 2) * P]
                gt = gp.tile([P, D], F32, tag="g")
                # 3 accumulating bf16 matmuls per half reconstruct the fp32
                # table row exactly: psum = ((hi + mid) + lo) == pe row.
                for t, (lhsT, sl) in enumerate(((ohy, 0), (ohx, 1))):
                    for k in range(3):
                        nc.tensor.matmul(
                            out=gt[:, sl * HALF : (sl + 1) * HALF],
                            lhsT=lhsT,
                            rhs=tabs[:, 3 * sl + k, :],
                            start=(k == 0),
                            stop=(k == 2),
                        )
                xt = xp.tile([P, D], F32)
                nc.sync.dma_start(out=xt[:], in_=x[c * P : (c + 1) * P, :])
                nc.vector.tensor_tensor(
                    out=xt[:], in0=xt[:], in1=gt[:], op=mybir.AluOpType.add
                )
                nc.scalar.dma_start(out=out[c * P : (c + 1) * P, :], in_=xt[:])


def build():
    nc = bacc.Bacc(
        "TRN2", target_bir_lowering=False, debug=False, enable_asserts=False
    )
    x = nc.dram_tensor("x", [S, D], F32, kind="ExternalInput").ap()
    boxes = nc.dram_tensor("boxes", [S, 4], F32, kind="ExternalInput").ap()
    ident_d = nc.dram_tensor("ident", [P, P], F32, kind="ExternalInput").ap()
    tlo_d = nc.dram_tensor("tlo", [NCLASS, 1], F32, kind="ExternalInput").ap()
    thi_d = nc.dram_tensor("thi", [NCLASS, 1], F32, kind="ExternalInput").ap()
    tabs_d = nc.dram_tensor(
        "tabs", [NCLASS, 6, HALF], BF16, kind="ExternalInput"
    ).ap()
    out = nc.dram_tensor("out", [S, D], F32, kind="ExternalOutput").ap()
    with TileContext(nc) as tc:
        _emit(tc, x, boxes, ident_d, tlo_d, thi_d, tabs_d, out)
    nc.compile()
    return nc


def _split3_bf16(pe):
    """Split fp32 [100, 512] into 3 bf16 planes with hi+mid+lo == pe exactly."""
    pe = np.asarray(pe, dtype=np.float32)
    hi = pe.astype(ml_dtypes.bfloat16)
    r1 = pe - hi.astype(np.float32)
    mid = r1.astype(ml_dtypes.bfloat16)
    r2 = r1 - mid.astype(np.float32)
    lo = r2.astype(ml_dtypes.bfloat16)
    recon = hi.astype(np.float32) + mid.astype(np.float32)
    recon = recon + lo.astype(np.float32)
    assert np.array_equal(recon, pe), "3-way bf16 split is not exact"
    return hi, mid, lo


def const_inputs(pe_h, pe_w):
    """Host-side constant tensors fed to every core."""
    ident = np.eye(P, dtype=np.float32)
    # RNE interval partition: T[k] = k - 0.5 (even k) / nextafter(k - 0.5)
    # (odd k); row k of the one-hot selects v in [T[k], T[k+1]).
    T = np.empty(NCLASS + 1, dtype=np.float32)
    for k in range(NCLASS + 1):
        t = np.float32(k) - np.float32(0.5)
        if k % 2 == 1:
            t = np.nextafter(t, np.float32(np.inf), dtype=np.float32)
        T[k] = t
    T[NCLASS] = np.float32(1e30)  # v can never reach row-100 territory
    tabs = np.stack(list(_split3_bf16(pe_h)) + list(_split3_bf16(pe_w)), axis=1)
    return {
        "ident": ident,
        "tlo": np.ascontiguousarray(T[0:NCLASS, None]),
        "thi": np.ascontiguousarray(T[1 : NCLASS + 1, None]),
        "tabs": np.ascontiguousarray(tabs),
    }


def _get_nc():
    global _NC
    if _NC is None:
        _NC = build()
    return _NC


def kernel(x, boxes, pe_h, pe_w):
    x = np.ascontiguousarray(np.asarray(x, dtype=np.float32))
    boxes = np.ascontiguousarray(np.asarray(boxes, dtype=np.float32))
    pe_h = np.ascontiguousarray(np.asarray(pe_h, dtype=np.float32))
    pe_w = np.ascontiguousarray(np.asarray(pe_w, dtype=np.float32))
    assert x.shape == (B, S, D) and boxes.shape == (B, S, 4)

    nc = _get_nc()
    consts = const_inputs(pe_h, pe_w)
    in_maps = [{"x": x[b], "boxes": boxes[b], **consts} for b in range(B)]
    trace = bool(os.environ.get("PE2D_TRACE"))
    res = run_bass_kernel_spmd(nc, in_maps, core_ids=list(range(B)), trace=trace)
    if trace:
        kernel.last_exec_time_ns = res.exec_time_ns
        kernel.last_mean_exec_time_ns = res.mean_exec_time_ns
        kernel.last_trace = res.instructions_and_trace
    return np.stack([r["out"] for r in res.results])


# revision 16
# speedup vs baseline: 1.1148x; 1.1148x over previous
"""Trainium2 Bass kernel: 2D positional encoding via embedding lookup.

Computes out[b, s, :] = x[b, s, :] + concat(pe_h[cy[b,s]], pe_w[cx[b,s]])
where cy/cx are integer grid indices derived from box centers (the TRN jax
backend's float->int cast rounds to nearest even; we reproduce that exactly
with precomputed per-row comparison thresholds).

Sharding: data-parallel over batch (8 batches -> 8 NeuronCores), pe tables
replicated. Per core the gather is done ON-CHIP: the [100, 512] tables live
in SBUF (split hi/mid/lo into three bf16 planes whose fp32 sum reconstructs
the table bit-exactly) and rows are selected with a one-hot bf16 matmul on
the tensor engine, so HBM traffic is just x-in + boxes + out.
"""

import os
import sys

for _p in ("/opt/trn_rl_repo", "/root/.axon_site/_ro/trn_rl_repo"):
    if os.path.isdir(_p) and _p not in sys.path:
        sys.path.insert(0, _p)

import ml_dtypes
import numpy as np

import concourse.bacc as bacc
import concourse.mybir as mybir
from concourse.bass_utils import run_bass_kernel_spmd
from concourse.tile import TileContext

B = 8
S = 4096
D = 1024
HALF = D // 2  # 512
NCLASS = 100
P = 128
NTILES = S // P  # 32
NPAIRS = NTILES // 2  # 16
F32 = mybir.dt.float32
BF16 = mybir.dt.bfloat16

_NC = None


def _emit(tc, x, boxes, ident_d, tlo_d, thi_d, tabs_d, out):
    nc = tc.nc
    with (
        tc.tile_pool(name="const", bufs=1) as const,
        tc.tile_pool(name="xp", bufs=8) as xp,
        tc.tile_pool(name="cmp", bufs=2) as cmp_pool,
        tc.tile_pool(name="bcp", bufs=2, space="PSUM") as bcp,
        tc.tile_pool(name="gp", bufs=3, space="PSUM") as gp,
    ):
        # boxes first on the sync queue (contiguous: [32 tiles, 128 tokens, 4]
        # with 2KB descriptors) so x-loads follow immediately; the remaining
        # constants ride the scalar queue, idle until the first out-store.
        bx = const.tile([NTILES, P, 4], F32)
        nc.sync.dma_start(out=bx[:], in_=boxes.rearrange("(c p) e -> c p e", c=NTILES))
        identity = const.tile([P, P], F32)
        nc.scalar.dma_start(out=identity[:], in_=ident_d)

        tlo = const.tile([NCLASS, 1], F32)
        nc.scalar.dma_start(out=tlo[:], in_=tlo_d)
        thi = const.tile([NCLASS, 1], F32)
        nc.scalar.dma_start(out=thi[:], in_=thi_d)

        # six bf16 table planes: [h_hi, h_mid, h_lo, w_hi, w_mid, w_lo]
        tabs = const.tile([NCLASS, 6, HALF], BF16)
        nc.scalar.dma_start(out=tabs[:], in_=tabs_d)

        # center values in row form: vrow[c, t] = v(token c*128+t), computed
        # as (lo + hi) * 49.5 -- same fp32 rounding as the reference's
        # ((lo + hi) * 0.5) * 99.0 (the *0.5 step is exact).
        vyr = const.tile([NTILES, P], F32)
        vxr = const.tile([NTILES, P], F32)
        nc.vector.tensor_tensor(
            out=vyr[:], in0=bx[:, :, 1], in1=bx[:, :, 3], op=mybir.AluOpType.add
        )
        nc.vector.tensor_scalar_mul(vyr[:], vyr[:], 49.5)
        nc.vector.tensor_tensor(
            out=vxr[:], in0=bx[:, :, 0], in1=bx[:, :, 2], op=mybir.AluOpType.add
        )
        nc.vector.tensor_scalar_mul(vxr[:], vxr[:], 49.5)

        # one-time transpose to column form: vcols[:, c] = vy col of tile c,
        # vcols[:, NTILES + c] = vx col of tile c.
        vt_ps = bcp.tile([P, 2 * NTILES], F32, tag="bc")
        nc.tensor.transpose(
            out=vt_ps[:, 0:NTILES], in_=vyr[:], identity=identity[0:NTILES, 0:NTILES]
        )
        nc.tensor.transpose(
            out=vt_ps[:, NTILES : 2 * NTILES],
            in_=vxr[:],
            identity=identity[0:NTILES, 0:NTILES],
        )
        vcols = const.tile([P, 2 * NTILES], F32)
        nc.vector.tensor_copy(vcols[:], vt_ps[:])

        for g in range(NPAIRS):
            c0 = 2 * g
            # Broadcast the pair's y/x values across NCLASS partitions via PE
            # transposes of free-dim-broadcast columns: bc[:, k*128:(k+1)*128]
            # = [vy(c0) | vx(c0) | vy(c0+1) | vx(c0+1)] broadcast rows.
            bc = bcp.tile([NCLASS, 4 * P], F32, tag="bc")
            for i in range(4):
                col = (c0 + i // 2) + (0 if i % 2 == 0 else NTILES)
                nc.tensor.transpose(
                    out=bc[:, i * P : (i + 1) * P],
                    in_=vcols[:, col : col + 1].to_broadcast([P, NCLASS]),
                    identity=identity[:],
                )
            # one-hot row k = (v >= tlo[k]) - (v >= thi[k]); exact {0,1} in bf16
            ge1 = cmp_pool.tile([NCLASS, 4 * P], F32, tag="ge1")
            nc.vector.tensor_scalar(
                out=ge1[:], in0=bc[:], scalar1=tlo[:], scalar2=None,
                op0=mybir.AluOpType.is_ge,
            )
            ge2 = cmp_pool.tile([NCLASS, 4 * P], F32, tag="ge2")
            nc.vector.tensor_scalar(
                out=ge2[:], in0=bc[:], scalar1=thi[:], scalar2=None,
                op0=mybir.AluOpType.is_ge,
            )
            oh = cmp_pool.tile([NCLASS, 4 * P], BF16, tag="oh")
            nc.vector.tensor_tensor(
                out=oh[:], in0=ge1[:], in1=ge2[:], op=mybir.AluOpType.subtract
            )

            for j in range(2):  # the two tiles of this pair
                c = c0 + j
                ohy = oh[:, (2 * j) * P : (2 * j + 1) * P]
                ohx = oh[:, (2 * j + 1) * P : (2 * j + 2) * P]
                gt = gp.tile([P, D], F32, tag="g")
                # 3 accumulating bf16 matmuls per half reconstruct the fp32
                # table row exactly: psum = ((hi + mid) + lo) == pe row.
                for t, (lhsT, sl) in enumerate(((ohy, 0), (ohx, 1))):
                    for k in range(3):
                        nc.tensor.matmul(
                            out=gt[:, sl * HALF : (sl + 1) * HALF],
                            lhsT=lhsT,
                            rhs=tabs[:, 3 * sl + k, :],
                            start=(k == 0),
                            stop=(k == 2),
                        )
                xt = xp.tile([P, D], F32)
                nc.sync.dma_start(out=xt[:], in_=x[c * P : (c + 1) * P, :])
                nc.vector.tensor_tensor(
                    out=xt[:], in0=xt[:], in1=gt[:], op=mybir.AluOpType.add
                )
                nc.scalar.dma_start(out=out[c * P : (c + 1) * P, :], in_=xt[:])


def build():
    nc = bacc.Bacc(
        "TRN2", target_bir_lowering=False, debug=False, enable_asserts=False
    )
    x = nc.dram_tensor("x", [S, D], F32, kind="ExternalInput").ap()
    boxes = nc.dram_tensor("boxes", [S, 4], F32, kind="ExternalInput").ap()
    ident_d = nc.dram_tensor("ident", [P, P], F32, kind="ExternalInput").ap()
    tlo_d = nc.dram_tensor("tlo", [NCLASS, 1], F32, kind="ExternalInput").ap()
    thi_d = nc.dram_tensor("thi", [NCLASS, 1], F32, kind="ExternalInput").ap()
    tabs_d = nc.dram_tensor(
        "tabs", [NCLASS, 6, HALF], BF16, kind="ExternalInput"
    ).ap()
    out = nc.dram_tensor("out", [S, D], F32, kind="ExternalOutput").ap()
    with TileContext(nc) as tc:
        _emit(tc, x, boxes, ident_d, tlo_d, thi_d, tabs_d, out)
    nc.compile()
    return nc


def _split3_bf16(pe):
    """Split fp32 [100, 512] into 3 bf16 planes with hi+mid+lo == pe exactly."""
    pe = np.asarray(pe, dtype=np.float32)
    hi = pe.astype(ml_dtypes.bfloat16)
    r1 = pe - hi.astype(np.float32)
    mid = r1.astype(ml_dtypes.bfloat16)
    r2 = r1 - mid.astype(np.float32)
    lo = r2.astype(ml_dtypes.bfloat16)
    recon = hi.astype(np.float32) + mid.astype(np.float32)
    recon = recon + lo.astype(np.float32)
    assert np.array_equal(recon, pe), "3-way bf16 split is not exact"
    return hi, mid, lo


def const_inputs(pe_h, pe_w):
    """Host-side constant tensors fed to every core."""
    ident = np.eye(P, dtype=np.float32)
    # RNE interval partition: T[k] = k - 0.5 (even k) / nextafter(k - 0.5)
    # (odd k); row k of the one-hot selects v in [T[k], T[k+1]).
    T = np.empty(NCLASS + 1, dtype=np.float32)
    for k in range(NCLASS + 1):
        t = np.float32(k) - np.float32(0.5)
        if k % 2 == 1:
            t = np.nextafter(t, np.float32(np.inf), dtype=np.float32)
        T[k] = t
    T[NCLASS] = np.float32(1e30)  # v can never reach row-100 territory
    tabs = np.stack(list(_split3_bf16(pe_h)) + list(_split3_bf16(pe_w)), axis=1)
    return {
        "ident": ident,
        "tlo": np.ascontiguousarray(T[0:NCLASS, None]),
        "thi": np.ascontiguousarray(T[1 : NCLASS + 1, None]),
        "tabs": np.ascontiguousarray(tabs),
    }


def _get_nc():
    global _NC
    if _NC is None:
        _NC = build()
    return _NC


def kernel(x, boxes, pe_h, pe_w):
    x = np.ascontiguousarray(np.asarray(x, dtype=np.float32))
    boxes = np.ascontiguousarray(np.asarray(boxes, dtype=np.float32))
    pe_h = np.ascontiguousarray(np.asarray(pe_h, dtype=np.float32))
    pe_w = np.ascontiguousarray(np.asarray(pe_w, dtype=np.float32))
    assert x.shape == (B, S, D) and boxes.shape == (B, S, 4)

    nc = _get_nc()
    consts = const_inputs(pe_h, pe_w)
    in_maps = [{"x": x[b], "boxes": boxes[b], **consts} for b in range(B)]
    trace = bool(os.environ.get("PE2D_TRACE"))
    res = run_bass_kernel_spmd(nc, in_maps, core_ids=list(range(B)), trace=trace)
    if trace:
        kernel.last_exec_time_ns = res.exec_time_ns
        kernel.last_mean_exec_time_ns = res.mean_exec_time_ns
        kernel.last_trace = res.instructions_and_trace
    return np.stack([r["out"] for r in res.results])
